# revision 1
# baseline (speedup 1.0000x reference)
"""MinkowskiInstanceNorm (segment-reduce instance norm) on 8 Trainium2 cores.

Strategy: seg_ids are sorted, so each segment is a contiguous run of rows.
With num_segments == n_cores == 8, core j owns segment j outright: it
computes sum(x) and sum(x^2) over its rows (padded to a fixed tile count
with zeros so one SPMD program serves all cores), derives
mean / inv_std / affine on-device, and normalizes in a second streaming
pass.  No cross-core communication is needed; the host only slices rows
per segment and stitches the outputs back in order.

Layout trick: the host packs each core's slab with CHANNELS ON
PARTITIONS — partition p = rb*32 + c (rb = row-block 0..3, c = channel),
free axis = 2048 consecutive rows of that block, i.e. x[T, 128, 2048].
Consequences on-device, per 1 MiB tile:
  - per-channel sums are plain full-free reductions ([128,2048]->[128,1]):
    one DVE tensor_reduce for sum(x), one ACT Square-activation whose
    accum_out gives sum(x^2) for free;
  - the normalization is ONE single-src DVE tensor_scalar
    (out = x*A[p] + B[p]) with per-partition scalars;
  - cross-partition folding (4 row-blocks per channel) is a tiny
    [128]x[128,2] matmul against a 0/1 selector, and the A/B broadcast
    back to 128 partitions is the transposed selector matmul.
PE and GpSimd stay idle; DVE+ACT total ~40% of the DMA streaming time,
so the kernel is HBM-bound as the problem demands.
"""

from contextlib import ExitStack

import numpy as np

C = 32  # channels
P = 128  # SBUF partitions
RB = P // C  # row blocks per tile (4)
FD = 2048  # rows per partition per tile (free dim)
ROWS = RB * FD  # rows per tile (8192)
NCORES = 8
EPS = 1e-8

_PROGRAMS = {}


def _emit(nc, tc, ctx, x_d, invn_d, w_d, b_d, s128_d, s32_d, o_d, T):
    from concourse import mybir

    dt = mybir.dt
    AX = mybir.AxisListType
    OP = mybir.AluOpType
    AF = mybir.ActivationFunctionType

    xv = x_d.ap()  # [T, P, FD]
    ov = o_d.ap()

    const = ctx.enter_context(tc.tile_pool(name="const", bufs=1))
    xpool = ctx.enter_context(tc.tile_pool(name="xpool", bufs=3))
    sqpool = ctx.enter_context(tc.tile_pool(name="sqpool", bufs=1))
    ypool = ctx.enter_context(tc.tile_pool(name="ypool", bufs=2))
    opool = ctx.enter_context(tc.tile_pool(name="opool", bufs=3))
    psum = ctx.enter_context(tc.tile_pool(name="psum", bufs=1, space="PSUM"))

    # First RES tiles stay resident in SBUF across both passes, skipping
    # their pass-2 reload (saves RES MiB of HBM reads per core).
    RES = min(14, T)
    res = const.tile([P, RES * FD], dt.float32)
    # single scratch for the (unread) Square outputs; ACT serializes anyway
    sqscratch = sqpool.tile([P, FD], dt.float32)

    invn = const.tile([C, 1], dt.float32)
    nc.sync.dma_start(out=invn[:], in_=invn_d.ap())
    wt = const.tile([C, 1], dt.float32)
    nc.sync.dma_start(out=wt[:], in_=w_d.ap())
    bt = const.tile([C, 1], dt.float32)
    nc.sync.dma_start(out=bt[:], in_=b_d.ap())
    sel128 = const.tile([P, C], dt.float32)
    nc.sync.dma_start(out=sel128[:], in_=s128_d.ap())
    sel32 = const.tile([C, P], dt.float32)
    nc.sync.dma_start(out=sel32[:], in_=s32_d.ap())

    sparts = const.tile([P, T], dt.float32)
    qparts = const.tile([P, T], dt.float32)

    for i in range(T):
        if i < RES:
            xt = res[:, i * FD : (i + 1) * FD]
        else:
            xt = xpool.tile([P, FD], dt.float32, tag="xt")
        nc.sync.dma_start(out=xt[:], in_=xv[i])
        nc.vector.tensor_reduce(
            out=sparts[:, i : i + 1], in_=xt[:], axis=AX.X, op=OP.add
        )
        nc.scalar.activation(
            sqscratch[:], xt[:], AF.Square, accum_out=qparts[:, i : i + 1]
        )

    st2 = const.tile([P, 2], dt.float32)
    nc.vector.tensor_reduce(out=st2[:, 0:1], in_=sparts[:], axis=AX.X, op=OP.add)
    nc.vector.tensor_reduce(out=st2[:, 1:2], in_=qparts[:], axis=AX.X, op=OP.add)

    # fold the RB row-blocks of each channel: [32, 2] = sel128.T @ st2
    tot = psum.tile([C, 2], dt.float32)
    nc.tensor.matmul(tot[:], lhsT=sel128[:], rhs=st2[:], start=True, stop=True)

    mean = const.tile([C, 1], dt.float32)
    nc.vector.tensor_scalar_mul(mean[:], tot[:, 0:1], invn[:])
    ex2 = const.tile([C, 1], dt.float32)
    nc.vector.tensor_scalar_mul(ex2[:], tot[:, 1:2], invn[:])
    msq = const.tile([C, 1], dt.float32)
    nc.vector.tensor_mul(msq[:], mean[:], mean[:])
    var = const.tile([C, 1], dt.float32)
    nc.vector.tensor_sub(var[:], ex2[:], msq[:])
    epsv = const.tile([C, 1], dt.float32)
    nc.vector.memset(epsv[:], EPS)
    std = const.tile([C, 1], dt.float32)
    nc.scalar.activation(std[:], var[:], AF.Sqrt, bias=epsv[:])
    istd = const.tile([C, 1], dt.float32)
    nc.vector.reciprocal(istd[:], std[:])
    # ab = [A | B]: A = w/std, B = b - mean*A
    ab = const.tile([C, 2], dt.float32)
    nc.vector.tensor_mul(ab[:, 0:1], istd[:], wt[:])
    nc.vector.tensor_mul(ab[:, 1:2], mean[:], ab[:, 0:1])
    nc.vector.tensor_sub(ab[:, 1:2], bt[:], ab[:, 1:2])

    # broadcast A/B back to all 128 partitions: [128, 2] = sel32.T @ ab
    abps = psum.tile([P, 2], dt.float32)
    nc.tensor.matmul(abps[:], lhsT=sel32[:], rhs=ab[:], start=True, stop=True)
    ab128 = const.tile([P, 2], dt.float32)
    nc.scalar.copy(ab128[:], abps[:])

    for i in range(T):
        if i < RES:
            yt = res[:, i * FD : (i + 1) * FD]
        else:
            yt = ypool.tile([P, FD], dt.float32, tag="yt")
            nc.sync.dma_start(out=yt[:], in_=xv[i])
        ot = opool.tile([P, FD], dt.float32, tag="ot")
        nc.vector.tensor_scalar(
            out=ot[:],
            in0=yt[:],
            scalar1=ab128[:, 0:1],
            scalar2=ab128[:, 1:2],
            op0=OP.mult,
            op1=OP.add,
        )
        # stores ride the ACT HW-DGE ring so a store waiting on compute
        # never head-of-line-blocks the load FIFO on the sync ring
        nc.scalar.dma_start(out=ov[i], in_=ot[:])


def _get_program(T):
    if T in _PROGRAMS:
        return _PROGRAMS[T]
    import concourse.tile as tile
    from concourse import bacc, mybir

    dt = mybir.dt
    nc = bacc.Bacc(
        "TRN2",
        target_bir_lowering=False,
        debug=False,
        enable_asserts=False,
        num_devices=NCORES,
    )
    x_d = nc.dram_tensor("x", [T, P, FD], dt.float32, kind="ExternalInput")
    invn_d = nc.dram_tensor("invn", [C, 1], dt.float32, kind="ExternalInput")
    w_d = nc.dram_tensor("w", [C, 1], dt.float32, kind="ExternalInput")
    b_d = nc.dram_tensor("b", [C, 1], dt.float32, kind="ExternalInput")
    s128_d = nc.dram_tensor("sel128", [P, C], dt.float32, kind="ExternalInput")
    s32_d = nc.dram_tensor("sel32", [C, P], dt.float32, kind="ExternalInput")
    o_d = nc.dram_tensor("o", [T, P, FD], dt.float32, kind="ExternalOutput")

    with tile.TileContext(nc) as tc:
        with ExitStack() as ctx:
            _emit(nc, tc, ctx, x_d, invn_d, w_d, b_d, s128_d, s32_d, o_d, T)

    nc.finalize()
    _PROGRAMS[T] = nc
    return nc


def _pack(rows, T):
    """rows [n, C] -> [T, 128, FD]: partition rb*32+c holds rows
    [t*ROWS + rb*FD + j] of channel c at free index j; zero padded."""
    PAD = T * ROWS
    xp = np.zeros((PAD, C), dtype=np.float32)
    xp[: rows.shape[0]] = rows
    return np.ascontiguousarray(
        xp.reshape(T, RB, FD, C).transpose(0, 1, 3, 2).reshape(T, P, FD)
    )


def _unpack(slab, n):
    """[T, 128, FD] -> rows [n, C]."""
    T = slab.shape[0]
    return (
        slab.reshape(T, RB, C, FD).transpose(0, 1, 3, 2).reshape(T * ROWS, C)[:n]
    )


def kernel(feats, seg_ids, weight, bias, num_segments, **_):
    from concourse.bass_utils import run_bass_kernel_spmd

    feats = np.ascontiguousarray(np.asarray(feats), dtype=np.float32)
    seg = np.asarray(seg_ids)
    w = np.asarray(weight, dtype=np.float32).reshape(C, 1)
    b = np.asarray(bias, dtype=np.float32).reshape(C, 1)
    S = int(num_segments)
    N = feats.shape[0]

    assert (np.diff(seg) >= 0).all(), "seg_ids must be sorted"
    bounds = np.searchsorted(seg, np.arange(S + 1)).astype(np.int64)
    counts = np.diff(bounds)

    sel128 = np.ascontiguousarray(np.tile(np.eye(C, dtype=np.float32), (RB, 1)))
    sel32 = np.ascontiguousarray(sel128.T)

    out = np.empty((N, C), dtype=np.float32)
    for g0 in range(0, S, NCORES):
        gsegs = list(range(g0, min(g0 + NCORES, S)))
        maxc = max(int(counts[s]) for s in gsegs)
        T = max(1, -(-maxc // ROWS))
        nc = _get_program(T)
        in_maps = []
        for j in range(NCORES):
            n_j = 1
            if j < len(gsegs):
                s = gsegs[j]
                n_j = max(int(counts[s]), 1)
                rows = feats[bounds[s] : bounds[s + 1]]
            else:
                rows = np.zeros((0, C), dtype=np.float32)
            in_maps.append(
                {
                    "x": _pack(rows, T),
                    "invn": np.full((C, 1), 1.0 / n_j, dtype=np.float32),
                    "w": w,
                    "b": b,
                    "sel128": sel128,
                    "sel32": sel32,
                }
            )
        results = run_bass_kernel_spmd(nc, in_maps, list(range(NCORES))).results
        for j, s in enumerate(gsegs):
            out[bounds[s] : bounds[s + 1]] = _unpack(results[j]["o"], int(counts[s]))
    return out



# revision 3
# speedup vs baseline: 1.0425x; 1.0425x over previous
"""MinkowskiInstanceNorm (segment-reduce instance norm) on 8 Trainium2 cores.

Strategy: seg_ids are sorted, so each segment is a contiguous run of rows.
With num_segments == n_cores == 8, core j owns segment j outright: it
computes sum(x) and sum(x^2) over its rows (padded to a fixed tile count
with zeros so one SPMD program serves all cores), derives
mean / inv_std / affine on-device, and normalizes in a second streaming
pass.  No cross-core communication is needed; the host only slices rows
per segment and stitches the outputs back in order.

Layout trick: the host packs each core's slab with CHANNELS ON
PARTITIONS — partition p = rb*32 + c (rb = row-block 0..3, c = channel),
free axis = 2048 consecutive rows of that block, i.e. x[T, 128, 2048].
Consequences on-device, per 1 MiB tile:
  - per-channel sums are plain full-free reductions ([128,2048]->[128,1]):
    one DVE tensor_reduce for sum(x), one ACT Square-activation whose
    accum_out gives sum(x^2) for free;
  - the normalization is ONE single-src DVE tensor_scalar
    (out = x*A[p] + B[p]) with per-partition scalars;
  - cross-partition folding (4 row-blocks per channel) is a tiny
    [128]x[128,2] matmul against a 0/1 selector, and the A/B broadcast
    back to 128 partitions is the transposed selector matmul.
PE and GpSimd stay idle; DVE+ACT total ~40% of the DMA streaming time,
so the kernel is HBM-bound as the problem demands.
"""

from contextlib import ExitStack

import numpy as np

C = 32  # channels
P = 128  # SBUF partitions
RB = P // C  # row blocks per tile (4)
FD = 2048  # rows per partition per tile (free dim)
ROWS = RB * FD  # rows per tile (8192)
NCORES = 8
EPS = 1e-8

_PROGRAMS = {}
LAST_RESULTS = None  # BassKernelResults of the most recent SPMD launch


def _emit(nc, tc, ctx, x_d, invn_d, w_d, b_d, s128_d, s32_d, o_d, T):
    from concourse import mybir

    dt = mybir.dt
    AX = mybir.AxisListType
    OP = mybir.AluOpType
    AF = mybir.ActivationFunctionType

    xv = x_d.ap()  # [T, P, FD]
    ov = o_d.ap()

    const = ctx.enter_context(tc.tile_pool(name="const", bufs=1))
    xpool = ctx.enter_context(tc.tile_pool(name="xpool", bufs=3))
    sqpool = ctx.enter_context(tc.tile_pool(name="sqpool", bufs=1))
    ypool = ctx.enter_context(tc.tile_pool(name="ypool", bufs=2))
    opool = ctx.enter_context(tc.tile_pool(name="opool", bufs=3))
    psum = ctx.enter_context(tc.tile_pool(name="psum", bufs=1, space="PSUM"))

    # First RES tiles stay resident in SBUF across both passes, skipping
    # their pass-2 reload (saves RES MiB of HBM reads per core).
    RES = min(14, T)
    res = const.tile([P, RES * FD], dt.float32)
    # single scratch for the (unread) Square outputs; ACT serializes anyway
    sqscratch = sqpool.tile([P, FD], dt.float32)

    invn = const.tile([C, 1], dt.float32)
    nc.sync.dma_start(out=invn[:], in_=invn_d.ap())
    wt = const.tile([C, 1], dt.float32)
    nc.sync.dma_start(out=wt[:], in_=w_d.ap())
    bt = const.tile([C, 1], dt.float32)
    nc.sync.dma_start(out=bt[:], in_=b_d.ap())
    sel128 = const.tile([P, C], dt.float32)
    nc.sync.dma_start(out=sel128[:], in_=s128_d.ap())
    sel32 = const.tile([C, P], dt.float32)
    nc.sync.dma_start(out=sel32[:], in_=s32_d.ap())

    sparts = const.tile([P, T], dt.float32)
    qparts = const.tile([P, T], dt.float32)

    for i in range(T):
        if i < RES:
            xt = res[:, i * FD : (i + 1) * FD]
        else:
            xt = xpool.tile([P, FD], dt.float32, tag="xt")
        nc.sync.dma_start(out=xt[:], in_=xv[i])
        nc.vector.tensor_reduce(
            out=sparts[:, i : i + 1], in_=xt[:], axis=AX.X, op=OP.add
        )
        nc.scalar.activation(
            sqscratch[:], xt[:], AF.Square, accum_out=qparts[:, i : i + 1]
        )

    st2 = const.tile([P, 2], dt.float32)
    nc.vector.tensor_reduce(out=st2[:, 0:1], in_=sparts[:], axis=AX.X, op=OP.add)
    nc.vector.tensor_reduce(out=st2[:, 1:2], in_=qparts[:], axis=AX.X, op=OP.add)

    # fold the RB row-blocks of each channel: [32, 2] = sel128.T @ st2
    tot = psum.tile([C, 2], dt.float32)
    nc.tensor.matmul(tot[:], lhsT=sel128[:], rhs=st2[:], start=True, stop=True)

    mean = const.tile([C, 1], dt.float32)
    nc.vector.tensor_scalar_mul(mean[:], tot[:, 0:1], invn[:])
    ex2 = const.tile([C, 1], dt.float32)
    nc.vector.tensor_scalar_mul(ex2[:], tot[:, 1:2], invn[:])
    msq = const.tile([C, 1], dt.float32)
    nc.vector.tensor_mul(msq[:], mean[:], mean[:])
    var = const.tile([C, 1], dt.float32)
    nc.vector.tensor_sub(var[:], ex2[:], msq[:])
    epsv = const.tile([C, 1], dt.float32)
    nc.vector.memset(epsv[:], EPS)
    std = const.tile([C, 1], dt.float32)
    nc.scalar.activation(std[:], var[:], AF.Sqrt, bias=epsv[:])
    istd = const.tile([C, 1], dt.float32)
    nc.vector.reciprocal(istd[:], std[:])
    # ab = [A | B]: A = w/std, B = b - mean*A
    ab = const.tile([C, 2], dt.float32)
    nc.vector.tensor_mul(ab[:, 0:1], istd[:], wt[:])
    nc.vector.tensor_mul(ab[:, 1:2], mean[:], ab[:, 0:1])
    nc.vector.tensor_sub(ab[:, 1:2], bt[:], ab[:, 1:2])

    # broadcast A/B back to all 128 partitions: [128, 2] = sel32.T @ ab
    abps = psum.tile([P, 2], dt.float32)
    nc.tensor.matmul(abps[:], lhsT=sel32[:], rhs=ab[:], start=True, stop=True)
    ab128 = const.tile([P, 2], dt.float32)
    nc.scalar.copy(ab128[:], abps[:])

    for i in range(T):
        if i < RES:
            yt = res[:, i * FD : (i + 1) * FD]
        else:
            yt = ypool.tile([P, FD], dt.float32, tag="yt")
            nc.sync.dma_start(out=yt[:], in_=xv[i])
        ot = opool.tile([P, FD], dt.float32, tag="ot")
        nc.vector.tensor_scalar(
            out=ot[:],
            in0=yt[:],
            scalar1=ab128[:, 0:1],
            scalar2=ab128[:, 1:2],
            op0=OP.mult,
            op1=OP.add,
        )
        # stores ride the ACT HW-DGE ring so a store waiting on compute
        # never head-of-line-blocks the load FIFO on the sync ring
        nc.scalar.dma_start(out=ov[i], in_=ot[:])


def _get_program(T):
    if T in _PROGRAMS:
        return _PROGRAMS[T]
    import concourse.tile as tile
    from concourse import bacc, mybir

    dt = mybir.dt
    nc = bacc.Bacc(
        "TRN2",
        target_bir_lowering=False,
        debug=False,
        enable_asserts=False,
        num_devices=NCORES,
    )
    x_d = nc.dram_tensor("x", [T, P, FD], dt.float32, kind="ExternalInput")
    invn_d = nc.dram_tensor("invn", [C, 1], dt.float32, kind="ExternalInput")
    w_d = nc.dram_tensor("w", [C, 1], dt.float32, kind="ExternalInput")
    b_d = nc.dram_tensor("b", [C, 1], dt.float32, kind="ExternalInput")
    s128_d = nc.dram_tensor("sel128", [P, C], dt.float32, kind="ExternalInput")
    s32_d = nc.dram_tensor("sel32", [C, P], dt.float32, kind="ExternalInput")
    o_d = nc.dram_tensor("o", [T, P, FD], dt.float32, kind="ExternalOutput")

    with tile.TileContext(nc) as tc:
        with ExitStack() as ctx:
            _emit(nc, tc, ctx, x_d, invn_d, w_d, b_d, s128_d, s32_d, o_d, T)

    nc.finalize()
    _PROGRAMS[T] = nc
    return nc


def _pack(rows, T):
    """rows [n, C] -> [T, 128, FD]: partition rb*32+c holds rows
    [t*ROWS + rb*FD + j] of channel c at free index j; zero padded."""
    PAD = T * ROWS
    xp = np.zeros((PAD, C), dtype=np.float32)
    xp[: rows.shape[0]] = rows
    return np.ascontiguousarray(
        xp.reshape(T, RB, FD, C).transpose(0, 1, 3, 2).reshape(T, P, FD)
    )


def _unpack(slab, n):
    """[T, 128, FD] -> rows [n, C]."""
    T = slab.shape[0]
    return (
        slab.reshape(T, RB, C, FD).transpose(0, 1, 3, 2).reshape(T * ROWS, C)[:n]
    )


def kernel(feats, seg_ids, weight, bias, num_segments, **_):
    from concourse.bass_utils import run_bass_kernel_spmd

    feats = np.ascontiguousarray(np.asarray(feats), dtype=np.float32)
    seg = np.asarray(seg_ids)
    w = np.asarray(weight, dtype=np.float32).reshape(C, 1)
    b = np.asarray(bias, dtype=np.float32).reshape(C, 1)
    S = int(num_segments)
    N = feats.shape[0]

    assert (np.diff(seg) >= 0).all(), "seg_ids must be sorted"
    bounds = np.searchsorted(seg, np.arange(S + 1)).astype(np.int64)
    counts = np.diff(bounds)

    sel128 = np.ascontiguousarray(np.tile(np.eye(C, dtype=np.float32), (RB, 1)))
    sel32 = np.ascontiguousarray(sel128.T)

    out = np.empty((N, C), dtype=np.float32)
    for g0 in range(0, S, NCORES):
        gsegs = list(range(g0, min(g0 + NCORES, S)))
        maxc = max(int(counts[s]) for s in gsegs)
        T = max(1, -(-maxc // ROWS))
        nc = _get_program(T)
        in_maps = []
        for j in range(NCORES):
            n_j = 1
            if j < len(gsegs):
                s = gsegs[j]
                n_j = max(int(counts[s]), 1)
                rows = feats[bounds[s] : bounds[s + 1]]
            else:
                rows = np.zeros((0, C), dtype=np.float32)
            in_maps.append(
                {
                    "x": _pack(rows, T),
                    "invn": np.full((C, 1), 1.0 / n_j, dtype=np.float32),
                    "w": w,
                    "b": b,
                    "sel128": sel128,
                    "sel32": sel32,
                }
            )
        r = run_bass_kernel_spmd(nc, in_maps, list(range(NCORES)))
        global LAST_RESULTS
        LAST_RESULTS = r
        results = r.results
        for j, s in enumerate(gsegs):
            out[bounds[s] : bounds[s + 1]] = _unpack(results[j]["o"], int(counts[s]))
    return out



# revision 7
# speedup vs baseline: 1.3345x; 1.2800x over previous
"""MinkowskiInstanceNorm (segment-reduce instance norm) on 8 Trainium2 cores.

Strategy: seg_ids are sorted, so each segment is a contiguous run of rows.
With num_segments == n_cores == 8, core j owns segment j outright: it
computes sum(x) and sum(x^2) over its rows (padded to a fixed tile count
with zeros so one SPMD program serves all cores), derives
mean / inv_std / affine on-device, and normalizes in a second pass.
No cross-core communication is needed; the host only slices rows
per segment and stitches the outputs back in order.

Perf design (HBM-bound problem):
  - The device slab is fp16 (host casts on pack, upcasts on unpack),
    halving HBM traffic vs fp32.  Quantization error ~5e-4 relative,
    far inside the 2e-2 gate.
  - The whole per-core slab (T tiles of [128, FD]) stays RESIDENT in
    SBUF across both passes, so pass 2 reads from SBUF: HBM traffic is
    exactly one read + one write of the data.
  - Layout: channels on partitions — partition p = rb*32 + c (rb =
    row-block 0..3, c = channel), free axis = FD consecutive rows of
    that block.  Per-channel sums are full-free reductions; the
    normalize is one single-src DVE tensor_scalar (4x perf mode on
    fp16); cross-partition folding (4 row-blocks per channel) is a tiny
    matmul against a 0/1 selector, broadcast back via its transpose.
  - Pass-1 sum(x^2) is split between DVE (tensor_tensor_reduce) and ACT
    (Square activation with accum_out) so neither engine throttles the
    load stream; sum(x) is one DVE tensor_reduce per tile.
  - Loads ride the sync HWDGE ring, stores the ACT HWDGE ring.
"""

from contextlib import ExitStack

import numpy as np

C = 32  # channels
P = 128  # SBUF partitions
RB = P // C  # row blocks per tile (4)
NCORES = 8
EPS = 1e-8
T_TARGET = 8  # preferred tile count (keeps per-DMA size ~2 MiB)
FD_CAP = 8192  # largest free dim per tile we allow when growing T instead

_PROGRAMS = {}
LAST_RESULTS = None  # BassKernelResults of the most recent SPMD launch


def _cfg(maxc):
    """Pick (T, FD) so T*RB*FD >= maxc with small padding waste."""
    need = max(int(maxc), 1)
    fd = -(-need // (T_TARGET * RB * 64)) * 64
    fd = max(fd, 512)
    if fd <= FD_CAP:
        return T_TARGET, fd
    return -(-need // (RB * FD_CAP)), FD_CAP


def _emit(nc, tc, ctx, x_d, invn_d, w_d, b_d, s128_d, s32_d, o_d, T, FD):
    from concourse import mybir

    dt = mybir.dt
    AX = mybir.AxisListType
    OP = mybir.AluOpType
    AF = mybir.ActivationFunctionType

    xv = x_d.ap()  # [T, P, FD]
    ov = o_d.ap()

    # ACT takes this many trailing elements of each tile's sum(x^2); DVE
    # (tensor_tensor_reduce) takes the rest.  Balanced so DVE ~ ACT ~ DMA.
    ACT_F = (int(FD * 0.36) // 32) * 32
    DVE_F = FD - ACT_F

    const = ctx.enter_context(tc.tile_pool(name="const", bufs=1))
    xpool = ctx.enter_context(tc.tile_pool(name="xpool", bufs=3))
    ypool = ctx.enter_context(tc.tile_pool(name="ypool", bufs=2))
    opool = ctx.enter_context(tc.tile_pool(name="opool", bufs=2))
    psum = ctx.enter_context(tc.tile_pool(name="psum", bufs=1, space="PSUM"))

    # Resident slab: as many tiles as fit in SBUF stay live across both
    # passes (all of them for the 2M-row / 8-segment problem).
    budget = 198 * 1024  # per-partition bytes, leaving slack for scratch
    fixed = (4 + 2 + 1) * FD * 2 + 4096  # opool+ypool+scratch+consts
    if T * FD * 2 + fixed > budget:
        RES = max((budget - fixed - 6 * FD * 2) // (FD * 2), 0)  # xpool too
        RES = min(T, int(RES))
    else:
        RES = T
    res = const.tile([P, RES * FD], dt.float16, name="res") if RES else None
    ttrscr = const.tile([P, DVE_F], dt.float16)
    sqscr = const.tile([P, ACT_F], dt.float16, name="sqscr") if ACT_F else None

    invn = const.tile([C, 1], dt.float32)
    nc.sync.dma_start(out=invn[:], in_=invn_d.ap())
    wt = const.tile([C, 1], dt.float32)
    nc.sync.dma_start(out=wt[:], in_=w_d.ap())
    bt = const.tile([C, 1], dt.float32)
    nc.sync.dma_start(out=bt[:], in_=b_d.ap())
    sel128 = const.tile([P, C], dt.float32)
    nc.sync.dma_start(out=sel128[:], in_=s128_d.ap())
    sel32 = const.tile([C, P], dt.float32)
    nc.sync.dma_start(out=sel32[:], in_=s32_d.ap())

    sparts = const.tile([P, T], dt.float32)
    qparts = const.tile([P, 2 * T], dt.float32)

    for i in range(T):
        if i < RES:
            xt = res[:, i * FD : (i + 1) * FD]
        else:
            xt = xpool.tile([P, FD], dt.float16, tag="xt")
        nc.sync.dma_start(out=xt[:], in_=xv[i])
        nc.vector.tensor_reduce(
            out=sparts[:, i : i + 1], in_=xt[:], axis=AX.X, op=OP.add
        )
        nc.vector.scalar_tensor_tensor(
            out=ttrscr[:],
            in0=xt[:, :DVE_F],
            scalar=1.0,
            in1=xt[:, :DVE_F],
            op0=OP.mult,
            op1=OP.mult,
            accum_out=qparts[:, i : i + 1],
        )
        if ACT_F:
            nc.scalar.activation(
                sqscr[:],
                xt[:, DVE_F:],
                AF.Square,
                accum_out=qparts[:, T + i : T + i + 1],
            )
        else:
            nc.vector.memset(qparts[:, T + i : T + i + 1], 0.0)

    st2 = const.tile([P, 2], dt.float32)
    nc.vector.tensor_reduce(out=st2[:, 0:1], in_=sparts[:], axis=AX.X, op=OP.add)
    nc.vector.tensor_reduce(out=st2[:, 1:2], in_=qparts[:], axis=AX.X, op=OP.add)

    # fold the RB row-blocks of each channel: [32, 2] = sel128.T @ st2
    tot = psum.tile([C, 2], dt.float32)
    nc.tensor.matmul(tot[:], lhsT=sel128[:], rhs=st2[:], start=True, stop=True)

    mean = const.tile([C, 1], dt.float32)
    nc.vector.tensor_scalar_mul(mean[:], tot[:, 0:1], invn[:])
    ex2 = const.tile([C, 1], dt.float32)
    nc.vector.tensor_scalar_mul(ex2[:], tot[:, 1:2], invn[:])
    msq = const.tile([C, 1], dt.float32)
    nc.vector.tensor_mul(msq[:], mean[:], mean[:])
    var = const.tile([C, 1], dt.float32)
    nc.vector.tensor_sub(var[:], ex2[:], msq[:])
    epsv = const.tile([C, 1], dt.float32)
    nc.vector.memset(epsv[:], EPS)
    std = const.tile([C, 1], dt.float32)
    nc.scalar.activation(std[:], var[:], AF.Sqrt, bias=epsv[:])
    istd = const.tile([C, 1], dt.float32)
    nc.vector.reciprocal(istd[:], std[:])
    # ab = [A | B]: A = w/std, B = b - mean*A
    ab = const.tile([C, 2], dt.float32)
    nc.vector.tensor_mul(ab[:, 0:1], istd[:], wt[:])
    nc.vector.tensor_mul(ab[:, 1:2], mean[:], ab[:, 0:1])
    nc.vector.tensor_sub(ab[:, 1:2], bt[:], ab[:, 1:2])

    # broadcast A/B back to all 128 partitions: [128, 2] = sel32.T @ ab
    abps = psum.tile([P, 2], dt.float32)
    nc.tensor.matmul(abps[:], lhsT=sel32[:], rhs=ab[:], start=True, stop=True)
    ab128 = const.tile([P, 2], dt.float32)
    nc.scalar.copy(ab128[:], abps[:])

    for i in range(T):
        if i < RES:
            yt = res[:, i * FD : (i + 1) * FD]
        else:
            yt = ypool.tile([P, FD], dt.float16, tag="yt")
            nc.sync.dma_start(out=yt[:], in_=xv[i])
        ot = opool.tile([P, FD], dt.float16, tag="ot")
        nc.vector.tensor_scalar(
            out=ot[:],
            in0=yt[:],
            scalar1=ab128[:, 0:1],
            scalar2=ab128[:, 1:2],
            op0=OP.mult,
            op1=OP.add,
        )
        # stores ride the ACT HW-DGE ring so they never head-of-line-block
        # the load FIFO on the sync ring
        nc.scalar.dma_start(out=ov[i], in_=ot[:])


def _get_program(T, FD):
    key = (T, FD)
    if key in _PROGRAMS:
        return _PROGRAMS[key]
    import concourse.tile as tile
    from concourse import bacc, mybir

    dt = mybir.dt
    nc = bacc.Bacc(
        "TRN2",
        target_bir_lowering=False,
        debug=False,
        enable_asserts=False,
        num_devices=NCORES,
    )
    x_d = nc.dram_tensor("x", [T, P, FD], dt.float16, kind="ExternalInput")
    invn_d = nc.dram_tensor("invn", [C, 1], dt.float32, kind="ExternalInput")
    w_d = nc.dram_tensor("w", [C, 1], dt.float32, kind="ExternalInput")
    b_d = nc.dram_tensor("b", [C, 1], dt.float32, kind="ExternalInput")
    s128_d = nc.dram_tensor("sel128", [P, C], dt.float32, kind="ExternalInput")
    s32_d = nc.dram_tensor("sel32", [C, P], dt.float32, kind="ExternalInput")
    o_d = nc.dram_tensor("o", [T, P, FD], dt.float16, kind="ExternalOutput")

    with tile.TileContext(nc) as tc:
        with ExitStack() as ctx:
            _emit(nc, tc, ctx, x_d, invn_d, w_d, b_d, s128_d, s32_d, o_d, T, FD)

    nc.finalize()
    _PROGRAMS[key] = nc
    return nc


def _pack(rows, T, FD):
    """rows [n, C] f32 -> [T, 128, FD] f16: partition rb*32+c holds rows
    [t*RB*FD + rb*FD + j] of channel c at free index j; zero padded."""
    PAD = T * RB * FD
    xp = np.zeros((PAD, C), dtype=np.float16)
    xp[: rows.shape[0]] = rows
    return np.ascontiguousarray(
        xp.reshape(T, RB, FD, C).transpose(0, 1, 3, 2).reshape(T, P, FD)
    )


def _unpack(slab, n):
    """[T, 128, FD] f16 -> rows [n, C] f32."""
    T, _, FD = slab.shape
    return (
        slab.reshape(T, RB, C, FD)
        .transpose(0, 1, 3, 2)
        .reshape(T * RB * FD, C)[:n]
        .astype(np.float32)
    )


def kernel(feats, seg_ids, weight, bias, num_segments, **_):
    from concourse.bass_utils import run_bass_kernel_spmd

    feats = np.ascontiguousarray(np.asarray(feats), dtype=np.float32)
    seg = np.asarray(seg_ids)
    w = np.asarray(weight, dtype=np.float32).reshape(C, 1)
    b = np.asarray(bias, dtype=np.float32).reshape(C, 1)
    S = int(num_segments)
    N = feats.shape[0]

    assert (np.diff(seg) >= 0).all(), "seg_ids must be sorted"
    bounds = np.searchsorted(seg, np.arange(S + 1)).astype(np.int64)
    counts = np.diff(bounds)

    sel128 = np.ascontiguousarray(np.tile(np.eye(C, dtype=np.float32), (RB, 1)))
    sel32 = np.ascontiguousarray(sel128.T)

    out = np.empty((N, C), dtype=np.float32)
    for g0 in range(0, S, NCORES):
        gsegs = list(range(g0, min(g0 + NCORES, S)))
        maxc = max(int(counts[s]) for s in gsegs)
        T, FD = _cfg(maxc)
        nc = _get_program(T, FD)
        in_maps = []
        for j in range(NCORES):
            n_j = 1
            if j < len(gsegs):
                s = gsegs[j]
                n_j = max(int(counts[s]), 1)
                rows = feats[bounds[s] : bounds[s + 1]]
            else:
                rows = np.zeros((0, C), dtype=np.float32)
            in_maps.append(
                {
                    "x": _pack(rows, T, FD),
                    "invn": np.full((C, 1), 1.0 / n_j, dtype=np.float32),
                    "w": w,
                    "b": b,
                    "sel128": sel128,
                    "sel32": sel32,
                }
            )
        r = run_bass_kernel_spmd(nc, in_maps, list(range(NCORES)))
        global LAST_RESULTS
        LAST_RESULTS = r
        results = r.results
        for j, s in enumerate(gsegs):
            out[bounds[s] : bounds[s + 1]] = _unpack(results[j]["o"], int(counts[s]))
    return out


# revision 9
# speedup vs baseline: 1.7358x; 1.3007x over previous
"""MinkowskiInstanceNorm (segment-reduce instance norm) on 8 Trainium2 cores.

Strategy: seg_ids are sorted, so each segment is a contiguous run of rows.
With num_segments == n_cores == 8, core j owns segment j outright: it
computes sum(x) and sum(x^2) over its rows (padded to a fixed tile count
with zeros so one SPMD program serves all cores), derives
mean / inv_std / affine on-device, and normalizes in a second pass.
No cross-core communication is needed; the host only slices rows
per segment and stitches the outputs back in order.

Perf design (HBM-bound problem):
  - The device slab is fp16 (host casts on pack, upcasts on unpack),
    halving HBM traffic vs fp32.  Quantization error ~5e-4 relative,
    far inside the 2e-2 gate.
  - The whole per-core slab (T tiles of [128, FD]) stays RESIDENT in
    SBUF across both passes, so pass 2 reads from SBUF: HBM traffic is
    exactly one read + one write of the data.
  - Layout: channels on partitions — partition p = rb*32 + c (rb =
    row-block 0..3, c = channel), free axis = FD consecutive rows of
    that block.  Per-channel sums are full-free reductions; the
    normalize is one single-src DVE tensor_scalar (4x perf mode on
    fp16); cross-partition folding (4 row-blocks per channel) is a tiny
    matmul against a 0/1 selector, broadcast back via its transpose.
  - Pass-1 sum(x^2) is split between DVE (tensor_tensor_reduce) and ACT
    (Square activation with accum_out) so neither engine throttles the
    load stream; sum(x) is one DVE tensor_reduce per tile.
  - Loads ride the sync HWDGE ring, stores the ACT HWDGE ring.
"""

from contextlib import ExitStack

import numpy as np

C = 32  # channels
P = 128  # SBUF partitions
RB = P // C  # row blocks per tile (4)
NCORES = 8
EPS = 1e-8
T_TARGET = 8  # preferred tile count (keeps per-DMA size ~2 MiB)
FD_CAP = 8192  # largest free dim per tile we allow when growing T instead

_PROGRAMS = {}
LAST_RESULTS = None  # BassKernelResults of the most recent SPMD launch


def _cfg(maxc):
    """Pick (T, FD) so T*RB*FD >= maxc with small padding waste."""
    need = max(int(maxc), 1)
    fd = -(-need // (T_TARGET * RB * 64)) * 64
    fd = max(fd, 512)
    if fd <= FD_CAP:
        return T_TARGET, fd
    return -(-need // (RB * FD_CAP)), FD_CAP


def _emit(nc, tc, ctx, x_d, invn_d, w_d, b_d, s128_d, s32_d, o_d, T, FD):
    from concourse import mybir

    dt = mybir.dt
    AX = mybir.AxisListType
    OP = mybir.AluOpType
    AF = mybir.ActivationFunctionType

    xv = x_d.ap()  # [T, P, FD]
    ov = o_d.ap()

    # Engine balance for pass-1 sum(x^2): ACT (Square+accum, ~1.35 ns/elem)
    # takes the trailing ACT_F elements, DVE (scalar_tensor_tensor, 1x mode
    # ~1.07 ns/elem) the rest.  DVE also owns sum(x) via a 2x-mode
    # tensor_tensor fold tree (~0.61 ns/elem), so the split solves
    # fold + 1.07*DVE_F == 1.35*ACT_F.
    ACT_F = (int(FD * 0.711) // 32) * 32
    DVE_F = FD - ACT_F

    const = ctx.enter_context(tc.tile_pool(name="const", bufs=1))
    xpool = ctx.enter_context(tc.tile_pool(name="xpool", bufs=3))
    ypool = ctx.enter_context(tc.tile_pool(name="ypool", bufs=2))
    opool = ctx.enter_context(tc.tile_pool(name="opool", bufs=2))
    psum = ctx.enter_context(tc.tile_pool(name="psum", bufs=1, space="PSUM"))

    # Resident slab: as many tiles as fit in SBUF stay live across both
    # passes (all of them for the 2M-row / 8-segment problem).
    budget = 196 * 1024  # per-partition bytes, leaving slack for scratch
    fixed = (2 + 1 + 2) * FD * 2 + 4096  # opool + scratches + consts
    if T * FD * 2 + fixed <= budget:
        RES = T
    else:
        RES = max((budget - fixed - 10 * FD) // (FD * 2), 0)  # xpool+ypool
        RES = min(T, int(RES))
    res = const.tile([P, RES * FD], dt.float16, name="res") if RES else None
    ttrscr = const.tile([P, DVE_F], dt.float16)
    sqscr = const.tile([P, ACT_F], dt.float16, name="sqscr") if ACT_F else None
    # ping-pong scratch for the sum(x) fold tree
    foldA = const.tile([P, FD // 2], dt.float16)
    foldB = const.tile([P, FD // 4], dt.float16)

    invn = const.tile([C, 1], dt.float32)
    nc.sync.dma_start(out=invn[:], in_=invn_d.ap())
    wt = const.tile([C, 1], dt.float32)
    nc.sync.dma_start(out=wt[:], in_=w_d.ap())
    bt = const.tile([C, 1], dt.float32)
    nc.sync.dma_start(out=bt[:], in_=b_d.ap())
    sel128 = const.tile([P, C], dt.float32)
    nc.sync.dma_start(out=sel128[:], in_=s128_d.ap())
    sel32 = const.tile([C, P], dt.float32)
    nc.sync.dma_start(out=sel32[:], in_=s32_d.ap())

    sparts = const.tile([P, T], dt.float32)
    qparts = const.tile([P, 2 * T], dt.float32)

    for i in range(T):
        if i < RES:
            xt = res[:, i * FD : (i + 1) * FD]
        else:
            xt = xpool.tile([P, FD], dt.float16, tag="xt")
        nc.sync.dma_start(out=xt[:], in_=xv[i])
        # sum(x): log2 fold tree of 2x-mode TT adds, then one tiny reduce.
        h = FD // 2
        nc.vector.tensor_add(foldA[:, :h], xt[:, :h], xt[:, h : 2 * h])
        src, dst = foldA, foldB
        while h > 256 and h % 2 == 0:
            h //= 2
            nc.vector.tensor_add(
                dst[:, :h], src[:, :h], src[:, h : 2 * h]
            )
            src, dst = dst, src
        nc.vector.tensor_reduce(
            out=sparts[:, i : i + 1], in_=src[:, :h], axis=AX.X, op=OP.add
        )
        nc.vector.scalar_tensor_tensor(
            out=ttrscr[:],
            in0=xt[:, :DVE_F],
            scalar=1.0,
            in1=xt[:, :DVE_F],
            op0=OP.mult,
            op1=OP.mult,
            accum_out=qparts[:, i : i + 1],
        )
        if ACT_F:
            nc.scalar.activation(
                sqscr[:],
                xt[:, DVE_F:],
                AF.Square,
                accum_out=qparts[:, T + i : T + i + 1],
            )
        else:
            nc.vector.memset(qparts[:, T + i : T + i + 1], 0.0)

    st2 = const.tile([P, 2], dt.float32)
    nc.vector.tensor_reduce(out=st2[:, 0:1], in_=sparts[:], axis=AX.X, op=OP.add)
    nc.vector.tensor_reduce(out=st2[:, 1:2], in_=qparts[:], axis=AX.X, op=OP.add)

    # fold the RB row-blocks of each channel: [32, 2] = sel128.T @ st2
    tot = psum.tile([C, 2], dt.float32)
    nc.tensor.matmul(tot[:], lhsT=sel128[:], rhs=st2[:], start=True, stop=True)

    mean = const.tile([C, 1], dt.float32)
    nc.vector.tensor_scalar_mul(mean[:], tot[:, 0:1], invn[:])
    ex2 = const.tile([C, 1], dt.float32)
    nc.vector.tensor_scalar_mul(ex2[:], tot[:, 1:2], invn[:])
    msq = const.tile([C, 1], dt.float32)
    nc.vector.tensor_mul(msq[:], mean[:], mean[:])
    var = const.tile([C, 1], dt.float32)
    nc.vector.tensor_sub(var[:], ex2[:], msq[:])
    epsv = const.tile([C, 1], dt.float32)
    nc.vector.memset(epsv[:], EPS)
    std = const.tile([C, 1], dt.float32)
    nc.scalar.activation(std[:], var[:], AF.Sqrt, bias=epsv[:])
    istd = const.tile([C, 1], dt.float32)
    nc.vector.reciprocal(istd[:], std[:])
    # ab = [A | B]: A = w/std, B = b - mean*A
    ab = const.tile([C, 2], dt.float32)
    nc.vector.tensor_mul(ab[:, 0:1], istd[:], wt[:])
    nc.vector.tensor_mul(ab[:, 1:2], mean[:], ab[:, 0:1])
    nc.vector.tensor_sub(ab[:, 1:2], bt[:], ab[:, 1:2])

    # broadcast A/B back to all 128 partitions: [128, 2] = sel32.T @ ab
    abps = psum.tile([P, 2], dt.float32)
    nc.tensor.matmul(abps[:], lhsT=sel32[:], rhs=ab[:], start=True, stop=True)
    ab128 = const.tile([P, 2], dt.float32)
    nc.scalar.copy(ab128[:], abps[:])

    for i in range(T):
        if i < RES:
            yt = res[:, i * FD : (i + 1) * FD]
        else:
            yt = ypool.tile([P, FD], dt.float16, tag="yt")
            nc.sync.dma_start(out=yt[:], in_=xv[i])
        ot = opool.tile([P, FD], dt.float16, tag="ot")
        nc.vector.tensor_scalar(
            out=ot[:],
            in0=yt[:],
            scalar1=ab128[:, 0:1],
            scalar2=ab128[:, 1:2],
            op0=OP.mult,
            op1=OP.add,
        )
        # stores ride the ACT HW-DGE ring so they never head-of-line-block
        # the load FIFO on the sync ring
        nc.scalar.dma_start(out=ov[i], in_=ot[:])


def _get_program(T, FD):
    key = (T, FD)
    if key in _PROGRAMS:
        return _PROGRAMS[key]
    import concourse.tile as tile
    from concourse import bacc, mybir

    dt = mybir.dt
    nc = bacc.Bacc(
        "TRN2",
        target_bir_lowering=False,
        debug=False,
        enable_asserts=False,
        num_devices=NCORES,
    )
    x_d = nc.dram_tensor("x", [T, P, FD], dt.float16, kind="ExternalInput")
    invn_d = nc.dram_tensor("invn", [C, 1], dt.float32, kind="ExternalInput")
    w_d = nc.dram_tensor("w", [C, 1], dt.float32, kind="ExternalInput")
    b_d = nc.dram_tensor("b", [C, 1], dt.float32, kind="ExternalInput")
    s128_d = nc.dram_tensor("sel128", [P, C], dt.float32, kind="ExternalInput")
    s32_d = nc.dram_tensor("sel32", [C, P], dt.float32, kind="ExternalInput")
    o_d = nc.dram_tensor("o", [T, P, FD], dt.float16, kind="ExternalOutput")

    with tile.TileContext(nc) as tc:
        with ExitStack() as ctx:
            _emit(nc, tc, ctx, x_d, invn_d, w_d, b_d, s128_d, s32_d, o_d, T, FD)

    nc.finalize()
    _PROGRAMS[key] = nc
    return nc


def _pack(rows, T, FD):
    """rows [n, C] f32 -> [T, 128, FD] f16: partition rb*32+c holds rows
    [t*RB*FD + rb*FD + j] of channel c at free index j; zero padded."""
    PAD = T * RB * FD
    xp = np.zeros((PAD, C), dtype=np.float16)
    xp[: rows.shape[0]] = rows
    return np.ascontiguousarray(
        xp.reshape(T, RB, FD, C).transpose(0, 1, 3, 2).reshape(T, P, FD)
    )


def _unpack(slab, n):
    """[T, 128, FD] f16 -> rows [n, C] f32."""
    T, _, FD = slab.shape
    return (
        slab.reshape(T, RB, C, FD)
        .transpose(0, 1, 3, 2)
        .reshape(T * RB * FD, C)[:n]
        .astype(np.float32)
    )


def kernel(feats, seg_ids, weight, bias, num_segments, **_):
    from concourse.bass_utils import run_bass_kernel_spmd

    feats = np.ascontiguousarray(np.asarray(feats), dtype=np.float32)
    seg = np.asarray(seg_ids)
    w = np.asarray(weight, dtype=np.float32).reshape(C, 1)
    b = np.asarray(bias, dtype=np.float32).reshape(C, 1)
    S = int(num_segments)
    N = feats.shape[0]

    assert (np.diff(seg) >= 0).all(), "seg_ids must be sorted"
    bounds = np.searchsorted(seg, np.arange(S + 1)).astype(np.int64)
    counts = np.diff(bounds)

    sel128 = np.ascontiguousarray(np.tile(np.eye(C, dtype=np.float32), (RB, 1)))
    sel32 = np.ascontiguousarray(sel128.T)

    out = np.empty((N, C), dtype=np.float32)
    for g0 in range(0, S, NCORES):
        gsegs = list(range(g0, min(g0 + NCORES, S)))
        maxc = max(int(counts[s]) for s in gsegs)
        T, FD = _cfg(maxc)
        nc = _get_program(T, FD)
        in_maps = []
        for j in range(NCORES):
            n_j = 1
            if j < len(gsegs):
                s = gsegs[j]
                n_j = max(int(counts[s]), 1)
                rows = feats[bounds[s] : bounds[s + 1]]
            else:
                rows = np.zeros((0, C), dtype=np.float32)
            in_maps.append(
                {
                    "x": _pack(rows, T, FD),
                    "invn": np.full((C, 1), 1.0 / n_j, dtype=np.float32),
                    "w": w,
                    "b": b,
                    "sel128": sel128,
                    "sel32": sel32,
                }
            )
        r = run_bass_kernel_spmd(nc, in_maps, list(range(NCORES)))
        global LAST_RESULTS
        LAST_RESULTS = r
        results = r.results
        for j, s in enumerate(gsegs):
            out[bounds[s] : bounds[s + 1]] = _unpack(results[j]["o"], int(counts[s]))
    return out


# revision 11
# speedup vs baseline: 2.1991x; 1.2669x over previous
"""MinkowskiInstanceNorm (segment-reduce instance norm) on 8 Trainium2 cores.

Strategy: seg_ids are sorted, so each segment is a contiguous run of rows.
With num_segments == n_cores == 8, core j owns segment j outright: it
computes sum(x) and sum(x^2) over its rows (padded to a fixed tile count
with zeros so one SPMD program serves all cores), derives
mean / inv_std / affine on-device, and normalizes in a second pass.
No cross-core communication is needed; the host only slices rows
per segment and stitches the outputs back in order.

Perf design (HBM-bound problem):
  - The device slab is fp16 (host casts on pack, upcasts on unpack),
    halving HBM traffic vs fp32.  Quantization error ~5e-4 relative,
    far inside the 2e-2 gate.
  - The whole per-core slab (T tiles of [128, FD]) stays RESIDENT in
    SBUF across both passes, so pass 2 reads from SBUF: HBM traffic is
    exactly one read + one write of the data.
  - Layout: channels on partitions — partition p = rb*32 + c (rb =
    row-block 0..3, c = channel), free axis = FD consecutive rows of
    that block.  Per-channel sums are full-free reductions; the
    normalize is one single-src DVE tensor_scalar (4x perf mode on
    fp16); cross-partition folding (4 row-blocks per channel) is a tiny
    matmul against a 0/1 selector, broadcast back via its transpose.
  - Pass-1 sum(x^2) is split between DVE (tensor_tensor_reduce) and ACT
    (Square activation with accum_out) so neither engine throttles the
    load stream; sum(x) is one DVE tensor_reduce per tile.
  - Loads ride the sync HWDGE ring, stores the ACT HWDGE ring.
"""

from contextlib import ExitStack

import numpy as np

C = 32  # channels
P = 128  # SBUF partitions
RB = P // C  # row blocks per tile (4)
NCORES = 8
EPS = 1e-8
T_TARGET = 8  # preferred tile count (keeps per-DMA size ~2 MiB)
FD_CAP = 8192  # largest free dim per tile we allow when growing T instead

_PROGRAMS = {}
LAST_RESULTS = None  # BassKernelResults of the most recent SPMD launch


def _cfg(maxc):
    """Pick (T, FD) so T*RB*FD >= maxc with small padding waste."""
    need = max(int(maxc), 1)
    fd = -(-need // (T_TARGET * RB * 64)) * 64
    fd = max(fd, 512)
    if fd <= FD_CAP:
        return T_TARGET, fd
    return -(-need // (RB * FD_CAP)), FD_CAP


def _emit(nc, tc, ctx, x_d, invn_d, w_d, b_d, s128_d, s32_d, o_d, T, FD):
    from concourse import mybir

    dt = mybir.dt
    AX = mybir.AxisListType
    OP = mybir.AluOpType
    AF = mybir.ActivationFunctionType

    xv = x_d.ap()  # [T, P, FD]
    ov = o_d.ap()

    # Engine balance for pass-1 sum(x^2): ACT (Square+accum, ~0.886 ns/elem
    # measured) takes the trailing ACT_F elements, DVE
    # (scalar_tensor_tensor, 1x mode ~1.07 ns/elem) the rest.  DVE also
    # owns sum(x) via a 2x-mode tensor_tensor fold tree (~0.65 ns/elem
    # all-in), so the split solves fold + 1.07*DVE_F == 0.886*ACT_F.
    ACT_F = (int(FD * 0.878) // 32) * 32
    DVE_F = FD - ACT_F

    const = ctx.enter_context(tc.tile_pool(name="const", bufs=1))
    xpool = ctx.enter_context(tc.tile_pool(name="xpool", bufs=3))
    ypool = ctx.enter_context(tc.tile_pool(name="ypool", bufs=2))
    opool = ctx.enter_context(tc.tile_pool(name="opool", bufs=2))
    psum = ctx.enter_context(tc.tile_pool(name="psum", bufs=1, space="PSUM"))

    # Resident slab: as many tiles as fit in SBUF stay live across both
    # passes (all of them for the 2M-row / 8-segment problem).
    budget = 200 * 1024  # per-partition bytes
    # opool(2 bufs) + fold ping-pong (0.75*FD) + stt/ACT scratch (FD) + consts
    fixed = 4 * FD + FD + FD // 2 + 2 * FD + 1024
    if T * FD * 2 + fixed <= budget:
        RES = T
    else:
        RES = max((budget - fixed - 10 * FD) // (FD * 2), 0)  # xpool+ypool
        RES = min(T, int(RES))
    res = const.tile([P, RES * FD], dt.float16, name="res") if RES else None
    ttrscr = const.tile([P, DVE_F], dt.float16)
    sqscr = const.tile([P, ACT_F], dt.float16, name="sqscr") if ACT_F else None
    # ping-pong scratch for the sum(x) fold tree
    foldA = const.tile([P, FD // 2], dt.float16)
    foldB = const.tile([P, FD // 4], dt.float16)

    invn = const.tile([C, 1], dt.float32)
    nc.sync.dma_start(out=invn[:], in_=invn_d.ap())
    wt = const.tile([C, 1], dt.float32)
    nc.sync.dma_start(out=wt[:], in_=w_d.ap())
    bt = const.tile([C, 1], dt.float32)
    nc.sync.dma_start(out=bt[:], in_=b_d.ap())
    sel128 = const.tile([P, C], dt.float32)
    nc.sync.dma_start(out=sel128[:], in_=s128_d.ap())
    sel32 = const.tile([C, P], dt.float32)
    nc.sync.dma_start(out=sel32[:], in_=s32_d.ap())

    sparts = const.tile([P, T], dt.float32)
    qparts = const.tile([P, 2 * T], dt.float32)

    for i in range(T):
        if i < RES:
            xt = res[:, i * FD : (i + 1) * FD]
        else:
            xt = xpool.tile([P, FD], dt.float16, tag="xt")
        nc.sync.dma_start(out=xt[:], in_=xv[i])
        # sum(x): log2 fold tree of 2x-mode TT adds, then one tiny reduce.
        h = FD // 2
        nc.vector.tensor_add(foldA[:, :h], xt[:, :h], xt[:, h : 2 * h])
        src, dst = foldA, foldB
        while h > 256 and h % 2 == 0:
            h //= 2
            nc.vector.tensor_add(
                dst[:, :h], src[:, :h], src[:, h : 2 * h]
            )
            src, dst = dst, src
        nc.vector.tensor_reduce(
            out=sparts[:, i : i + 1], in_=src[:, :h], axis=AX.X, op=OP.add
        )
        nc.vector.scalar_tensor_tensor(
            out=ttrscr[:],
            in0=xt[:, :DVE_F],
            scalar=1.0,
            in1=xt[:, :DVE_F],
            op0=OP.mult,
            op1=OP.mult,
            accum_out=qparts[:, i : i + 1],
        )
        if ACT_F:
            nc.scalar.activation(
                sqscr[:],
                xt[:, DVE_F:],
                AF.Square,
                accum_out=qparts[:, T + i : T + i + 1],
            )
        else:
            nc.vector.memset(qparts[:, T + i : T + i + 1], 0.0)

    st2 = const.tile([P, 2], dt.float32)
    nc.vector.tensor_reduce(out=st2[:, 0:1], in_=sparts[:], axis=AX.X, op=OP.add)
    nc.vector.tensor_reduce(out=st2[:, 1:2], in_=qparts[:], axis=AX.X, op=OP.add)

    # fold the RB row-blocks of each channel: [32, 2] = sel128.T @ st2
    tot = psum.tile([C, 2], dt.float32)
    nc.tensor.matmul(tot[:], lhsT=sel128[:], rhs=st2[:], start=True, stop=True)

    mean = const.tile([C, 1], dt.float32)
    nc.vector.tensor_scalar_mul(mean[:], tot[:, 0:1], invn[:])
    ex2 = const.tile([C, 1], dt.float32)
    nc.vector.tensor_scalar_mul(ex2[:], tot[:, 1:2], invn[:])
    msq = const.tile([C, 1], dt.float32)
    nc.vector.tensor_mul(msq[:], mean[:], mean[:])
    var = const.tile([C, 1], dt.float32)
    nc.vector.tensor_sub(var[:], ex2[:], msq[:])
    epsv = const.tile([C, 1], dt.float32)
    nc.vector.memset(epsv[:], EPS)
    std = const.tile([C, 1], dt.float32)
    nc.scalar.activation(std[:], var[:], AF.Sqrt, bias=epsv[:])
    istd = const.tile([C, 1], dt.float32)
    nc.vector.reciprocal(istd[:], std[:])
    # ab = [A | B]: A = w/std, B = b - mean*A
    ab = const.tile([C, 2], dt.float32)
    nc.vector.tensor_mul(ab[:, 0:1], istd[:], wt[:])
    nc.vector.tensor_mul(ab[:, 1:2], mean[:], ab[:, 0:1])
    nc.vector.tensor_sub(ab[:, 1:2], bt[:], ab[:, 1:2])

    # broadcast A/B back to all 128 partitions: [128, 2] = sel32.T @ ab
    abps = psum.tile([P, 2], dt.float32)
    nc.tensor.matmul(abps[:], lhsT=sel32[:], rhs=ab[:], start=True, stop=True)
    ab128 = const.tile([P, 2], dt.float32)
    nc.scalar.copy(ab128[:], abps[:])

    for i in range(T):
        if i < RES:
            yt = res[:, i * FD : (i + 1) * FD]
        else:
            yt = ypool.tile([P, FD], dt.float16, tag="yt")
            nc.sync.dma_start(out=yt[:], in_=xv[i])
        ot = opool.tile([P, FD], dt.float16, tag="ot")
        nc.vector.tensor_scalar(
            out=ot[:],
            in0=yt[:],
            scalar1=ab128[:, 0:1],
            scalar2=ab128[:, 1:2],
            op0=OP.mult,
            op1=OP.add,
        )
        # stores ride the ACT HW-DGE ring so they never head-of-line-block
        # the load FIFO on the sync ring
        nc.scalar.dma_start(out=ov[i], in_=ot[:])


def _get_program(T, FD):
    key = (T, FD)
    if key in _PROGRAMS:
        return _PROGRAMS[key]
    import concourse.tile as tile
    from concourse import bacc, mybir

    dt = mybir.dt
    nc = bacc.Bacc(
        "TRN2",
        target_bir_lowering=False,
        debug=False,
        enable_asserts=False,
        num_devices=NCORES,
    )
    x_d = nc.dram_tensor("x", [T, P, FD], dt.float16, kind="ExternalInput")
    invn_d = nc.dram_tensor("invn", [C, 1], dt.float32, kind="ExternalInput")
    w_d = nc.dram_tensor("w", [C, 1], dt.float32, kind="ExternalInput")
    b_d = nc.dram_tensor("b", [C, 1], dt.float32, kind="ExternalInput")
    s128_d = nc.dram_tensor("sel128", [P, C], dt.float32, kind="ExternalInput")
    s32_d = nc.dram_tensor("sel32", [C, P], dt.float32, kind="ExternalInput")
    o_d = nc.dram_tensor("o", [T, P, FD], dt.float16, kind="ExternalOutput")

    with tile.TileContext(nc) as tc:
        with ExitStack() as ctx:
            _emit(nc, tc, ctx, x_d, invn_d, w_d, b_d, s128_d, s32_d, o_d, T, FD)

    nc.finalize()
    _PROGRAMS[key] = nc
    return nc


def _pack(rows, T, FD):
    """rows [n, C] f32 -> [T, 128, FD] f16: partition rb*32+c holds rows
    [t*RB*FD + rb*FD + j] of channel c at free index j; zero padded."""
    PAD = T * RB * FD
    xp = np.zeros((PAD, C), dtype=np.float16)
    xp[: rows.shape[0]] = rows
    return np.ascontiguousarray(
        xp.reshape(T, RB, FD, C).transpose(0, 1, 3, 2).reshape(T, P, FD)
    )


def _unpack(slab, n):
    """[T, 128, FD] f16 -> rows [n, C] f32."""
    T, _, FD = slab.shape
    return (
        slab.reshape(T, RB, C, FD)
        .transpose(0, 1, 3, 2)
        .reshape(T * RB * FD, C)[:n]
        .astype(np.float32)
    )


def kernel(feats, seg_ids, weight, bias, num_segments, **_):
    from concourse.bass_utils import run_bass_kernel_spmd

    feats = np.ascontiguousarray(np.asarray(feats), dtype=np.float32)
    seg = np.asarray(seg_ids)
    w = np.asarray(weight, dtype=np.float32).reshape(C, 1)
    b = np.asarray(bias, dtype=np.float32).reshape(C, 1)
    S = int(num_segments)
    N = feats.shape[0]

    assert (np.diff(seg) >= 0).all(), "seg_ids must be sorted"
    bounds = np.searchsorted(seg, np.arange(S + 1)).astype(np.int64)
    counts = np.diff(bounds)

    sel128 = np.ascontiguousarray(np.tile(np.eye(C, dtype=np.float32), (RB, 1)))
    sel32 = np.ascontiguousarray(sel128.T)

    out = np.empty((N, C), dtype=np.float32)
    for g0 in range(0, S, NCORES):
        gsegs = list(range(g0, min(g0 + NCORES, S)))
        maxc = max(int(counts[s]) for s in gsegs)
        T, FD = _cfg(maxc)
        nc = _get_program(T, FD)
        in_maps = []
        for j in range(NCORES):
            n_j = 1
            if j < len(gsegs):
                s = gsegs[j]
                n_j = max(int(counts[s]), 1)
                rows = feats[bounds[s] : bounds[s + 1]]
            else:
                rows = np.zeros((0, C), dtype=np.float32)
            in_maps.append(
                {
                    "x": _pack(rows, T, FD),
                    "invn": np.full((C, 1), 1.0 / n_j, dtype=np.float32),
                    "w": w,
                    "b": b,
                    "sel128": sel128,
                    "sel32": sel32,
                }
            )
        r = run_bass_kernel_spmd(nc, in_maps, list(range(NCORES)))
        global LAST_RESULTS
        LAST_RESULTS = r
        results = r.results
        for j, s in enumerate(gsegs):
            out[bounds[s] : bounds[s + 1]] = _unpack(results[j]["o"], int(counts[s]))
    return out


# revision 15
# speedup vs baseline: 2.2667x; 1.0307x over previous
"""MinkowskiInstanceNorm (segment-reduce instance norm) on 8 Trainium2 cores.

Strategy: seg_ids are sorted, so each segment is a contiguous run of rows.
With num_segments == n_cores == 8, core j owns segment j outright: it
computes sum(x) and sum(x^2) over its rows (padded to a fixed tile count
with zeros so one SPMD program serves all cores), derives
mean / inv_std / affine on-device, and normalizes in a second pass.
No cross-core communication is needed; the host only slices rows
per segment and stitches the outputs back in order.

Perf design (HBM-bound problem):
  - The device slab is fp16 (host casts on pack, upcasts on unpack),
    halving HBM traffic vs fp32.  Quantization error ~5e-4 relative,
    far inside the 2e-2 gate.
  - The whole per-core slab (T tiles of [128, FD]) stays RESIDENT in
    SBUF across both passes, so pass 2 reads from SBUF: HBM traffic is
    exactly one read + one write of the data.
  - Layout: channels on partitions — partition p = rb*32 + c (rb =
    row-block 0..3, c = channel), free axis = FD consecutive rows of
    that block.  Per-channel sums are full-free reductions; the
    normalize is one single-src DVE tensor_scalar (4x perf mode on
    fp16); cross-partition folding (4 row-blocks per channel) is a tiny
    matmul against a 0/1 selector, broadcast back via its transpose.
  - Pass-1 sum(x^2) is split between DVE (tensor_tensor_reduce) and ACT
    (Square activation with accum_out) so neither engine throttles the
    load stream; sum(x) is one DVE tensor_reduce per tile.
  - Loads ride the sync HWDGE ring, stores the ACT HWDGE ring.
"""

from contextlib import ExitStack

import numpy as np

C = 32  # channels
P = 128  # SBUF partitions
RB = P // C  # row blocks per tile (4)
NCORES = 8
EPS = 1e-8
T_TARGET = 8  # preferred tile count (keeps per-DMA size ~2 MiB)
FD_CAP = 8192  # largest free dim per tile we allow when growing T instead

_PROGRAMS = {}
LAST_RESULTS = None  # BassKernelResults of the most recent SPMD launch


def _cfg(maxc):
    """Pick (T, FD) so T*RB*FD >= maxc with small padding waste."""
    need = max(int(maxc), 1)
    fd = -(-need // (T_TARGET * RB * 64)) * 64
    fd = max(fd, 512)
    if fd <= FD_CAP:
        return T_TARGET, fd
    return -(-need // (RB * FD_CAP)), FD_CAP


def _emit(nc, tc, ctx, x_d, invn_d, w_d, b_d, s128_d, s32_d, o_d, T, FD):
    from concourse import mybir

    dt = mybir.dt
    AX = mybir.AxisListType
    OP = mybir.AluOpType
    AF = mybir.ActivationFunctionType

    xv = x_d.ap()  # [T, P, FD]
    ov = o_d.ap()

    # Engine balance for pass-1 sum(x^2): ACT (Square+accum, ~0.886 ns/elem
    # measured) takes the trailing ACT_F elements, DVE
    # (scalar_tensor_tensor, 1x mode ~1.07 ns/elem) the rest.  DVE also
    # owns sum(x) via a 2x-mode tensor_tensor fold tree (~0.65 ns/elem
    # all-in), so the split solves fold + 1.07*DVE_F == 0.886*ACT_F.
    ACT_F = (int(FD * 0.878) // 32) * 32
    DVE_F = FD - ACT_F

    const = ctx.enter_context(tc.tile_pool(name="const", bufs=1))
    xpool = ctx.enter_context(tc.tile_pool(name="xpool", bufs=3))
    ypool = ctx.enter_context(tc.tile_pool(name="ypool", bufs=2))
    opool = ctx.enter_context(tc.tile_pool(name="opool", bufs=3))
    psum = ctx.enter_context(tc.tile_pool(name="psum", bufs=1, space="PSUM"))

    # Resident slab: as many tiles as fit in SBUF stay live across both
    # passes (all of them for the 2M-row / 8-segment problem).
    budget = 200 * 1024  # per-partition bytes
    # opool(3 bufs) + fold ping-pong (0.75*FD) + stt/ACT scratch (FD) + consts
    fixed = 6 * FD + FD + FD // 2 + 2 * FD + 1024
    if T * FD * 2 + fixed <= budget:
        RES = T
    else:
        RES = max((budget - fixed - 10 * FD) // (FD * 2), 0)  # xpool+ypool
        RES = min(T, int(RES))
    res = const.tile([P, RES * FD], dt.float16, name="res") if RES else None
    ttrscr = const.tile([P, DVE_F], dt.float16)
    sqscr = const.tile([P, ACT_F], dt.float16, name="sqscr") if ACT_F else None
    # ping-pong scratch for the sum(x) fold tree
    foldA = const.tile([P, FD // 2], dt.float16)
    foldB = const.tile([P, FD // 4], dt.float16)

    sparts = const.tile([P, T], dt.float32)
    qparts = const.tile([P, 2 * T], dt.float32)

    # Issue every resident tile load first so the sync-ring FIFO streams
    # them back-to-back; the tiny const loads follow (needed only ~40us
    # later, at stats time).
    for i in range(RES):
        nc.sync.dma_start(out=res[:, i * FD : (i + 1) * FD], in_=xv[i])

    invn = const.tile([C, 1], dt.float32)
    nc.sync.dma_start(out=invn[:], in_=invn_d.ap())
    wt = const.tile([C, 1], dt.float32)
    nc.sync.dma_start(out=wt[:], in_=w_d.ap())
    bt = const.tile([C, 1], dt.float32)
    nc.sync.dma_start(out=bt[:], in_=b_d.ap())
    sel128 = const.tile([P, C], dt.float32)
    nc.sync.dma_start(out=sel128[:], in_=s128_d.ap())
    sel32 = const.tile([C, P], dt.float32)
    nc.sync.dma_start(out=sel32[:], in_=s32_d.ap())

    for i in range(T):
        if i < RES:
            xt = res[:, i * FD : (i + 1) * FD]
        else:
            xt = xpool.tile([P, FD], dt.float16, tag="xt")
            nc.sync.dma_start(out=xt[:], in_=xv[i])
        # sum(x): log2 fold tree of 2x-mode TT adds, then one tiny reduce.
        h = FD // 2
        nc.vector.tensor_add(foldA[:, :h], xt[:, :h], xt[:, h : 2 * h])
        src, dst = foldA, foldB
        while h > 256 and h % 2 == 0:
            h //= 2
            nc.vector.tensor_add(
                dst[:, :h], src[:, :h], src[:, h : 2 * h]
            )
            src, dst = dst, src
        nc.vector.tensor_reduce(
            out=sparts[:, i : i + 1], in_=src[:, :h], axis=AX.X, op=OP.add
        )
        nc.vector.scalar_tensor_tensor(
            out=ttrscr[:],
            in0=xt[:, :DVE_F],
            scalar=1.0,
            in1=xt[:, :DVE_F],
            op0=OP.mult,
            op1=OP.mult,
            accum_out=qparts[:, i : i + 1],
        )
        if ACT_F:
            nc.scalar.activation(
                sqscr[:],
                xt[:, DVE_F:],
                AF.Square,
                accum_out=qparts[:, T + i : T + i + 1],
            )
        else:
            nc.vector.memset(qparts[:, T + i : T + i + 1], 0.0)

    st2 = const.tile([P, 2], dt.float32)
    nc.vector.tensor_reduce(out=st2[:, 0:1], in_=sparts[:], axis=AX.X, op=OP.add)
    nc.vector.tensor_reduce(out=st2[:, 1:2], in_=qparts[:], axis=AX.X, op=OP.add)

    # fold the RB row-blocks of each channel: [32, 2] = sel128.T @ st2
    tot = psum.tile([C, 2], dt.float32)
    nc.tensor.matmul(tot[:], lhsT=sel128[:], rhs=st2[:], start=True, stop=True)

    mean = const.tile([C, 1], dt.float32)
    nc.vector.tensor_scalar_mul(mean[:], tot[:, 0:1], invn[:])
    ex2 = const.tile([C, 1], dt.float32)
    nc.vector.tensor_scalar_mul(ex2[:], tot[:, 1:2], invn[:])
    msq = const.tile([C, 1], dt.float32)
    nc.vector.tensor_mul(msq[:], mean[:], mean[:])
    var = const.tile([C, 1], dt.float32)
    nc.vector.tensor_sub(var[:], ex2[:], msq[:])
    epsv = const.tile([C, 1], dt.float32)
    nc.vector.memset(epsv[:], EPS)
    std = const.tile([C, 1], dt.float32)
    nc.scalar.activation(std[:], var[:], AF.Sqrt, bias=epsv[:])
    istd = const.tile([C, 1], dt.float32)
    nc.vector.reciprocal(istd[:], std[:])
    # ab = [A | B]: A = w/std, B = b - mean*A
    ab = const.tile([C, 2], dt.float32)
    nc.vector.tensor_mul(ab[:, 0:1], istd[:], wt[:])
    nc.vector.tensor_mul(ab[:, 1:2], mean[:], ab[:, 0:1])
    nc.vector.tensor_sub(ab[:, 1:2], bt[:], ab[:, 1:2])

    # broadcast A/B back to all 128 partitions: [128, 2] = sel32.T @ ab
    abps = psum.tile([P, 2], dt.float32)
    nc.tensor.matmul(abps[:], lhsT=sel32[:], rhs=ab[:], start=True, stop=True)
    ab128 = const.tile([P, 2], dt.float32)
    nc.scalar.copy(ab128[:], abps[:])

    for i in range(T):
        if i < RES:
            yt = res[:, i * FD : (i + 1) * FD]
        else:
            yt = ypool.tile([P, FD], dt.float16, tag="yt")
            nc.sync.dma_start(out=yt[:], in_=xv[i])
        ot = opool.tile([P, FD], dt.float16, tag="ot")
        nc.vector.tensor_scalar(
            out=ot[:],
            in0=yt[:],
            scalar1=ab128[:, 0:1],
            scalar2=ab128[:, 1:2],
            op0=OP.mult,
            op1=OP.add,
        )
        # alternate stores across both HWDGE rings (sync ring is idle in
        # pass 2) for better sustained store bandwidth
        if i % 2 == 0:
            nc.scalar.dma_start(out=ov[i], in_=ot[:])
        else:
            nc.sync.dma_start(out=ov[i], in_=ot[:])


def _get_program(T, FD):
    key = (T, FD)
    if key in _PROGRAMS:
        return _PROGRAMS[key]
    import concourse.tile as tile
    from concourse import bacc, mybir

    dt = mybir.dt
    nc = bacc.Bacc(
        "TRN2",
        target_bir_lowering=False,
        debug=False,
        enable_asserts=False,
        num_devices=NCORES,
    )
    x_d = nc.dram_tensor("x", [T, P, FD], dt.float16, kind="ExternalInput")
    invn_d = nc.dram_tensor("invn", [C, 1], dt.float32, kind="ExternalInput")
    w_d = nc.dram_tensor("w", [C, 1], dt.float32, kind="ExternalInput")
    b_d = nc.dram_tensor("b", [C, 1], dt.float32, kind="ExternalInput")
    s128_d = nc.dram_tensor("sel128", [P, C], dt.float32, kind="ExternalInput")
    s32_d = nc.dram_tensor("sel32", [C, P], dt.float32, kind="ExternalInput")
    o_d = nc.dram_tensor("o", [T, P, FD], dt.float16, kind="ExternalOutput")

    with tile.TileContext(nc) as tc:
        with ExitStack() as ctx:
            _emit(nc, tc, ctx, x_d, invn_d, w_d, b_d, s128_d, s32_d, o_d, T, FD)

    nc.finalize()
    _PROGRAMS[key] = nc
    return nc


def _pack(rows, T, FD):
    """rows [n, C] f32 -> [T, 128, FD] f16: partition rb*32+c holds rows
    [t*RB*FD + rb*FD + j] of channel c at free index j; zero padded."""
    PAD = T * RB * FD
    xp = np.zeros((PAD, C), dtype=np.float16)
    xp[: rows.shape[0]] = rows
    return np.ascontiguousarray(
        xp.reshape(T, RB, FD, C).transpose(0, 1, 3, 2).reshape(T, P, FD)
    )


def _unpack(slab, n):
    """[T, 128, FD] f16 -> rows [n, C] f32."""
    T, _, FD = slab.shape
    return (
        slab.reshape(T, RB, C, FD)
        .transpose(0, 1, 3, 2)
        .reshape(T * RB * FD, C)[:n]
        .astype(np.float32)
    )


def kernel(feats, seg_ids, weight, bias, num_segments, **_):
    from concourse.bass_utils import run_bass_kernel_spmd

    feats = np.ascontiguousarray(np.asarray(feats), dtype=np.float32)
    seg = np.asarray(seg_ids)
    w = np.asarray(weight, dtype=np.float32).reshape(C, 1)
    b = np.asarray(bias, dtype=np.float32).reshape(C, 1)
    S = int(num_segments)
    N = feats.shape[0]

    assert (np.diff(seg) >= 0).all(), "seg_ids must be sorted"
    bounds = np.searchsorted(seg, np.arange(S + 1)).astype(np.int64)
    counts = np.diff(bounds)

    sel128 = np.ascontiguousarray(np.tile(np.eye(C, dtype=np.float32), (RB, 1)))
    sel32 = np.ascontiguousarray(sel128.T)

    out = np.empty((N, C), dtype=np.float32)
    for g0 in range(0, S, NCORES):
        gsegs = list(range(g0, min(g0 + NCORES, S)))
        maxc = max(int(counts[s]) for s in gsegs)
        T, FD = _cfg(maxc)
        nc = _get_program(T, FD)
        in_maps = []
        for j in range(NCORES):
            n_j = 1
            if j < len(gsegs):
                s = gsegs[j]
                n_j = max(int(counts[s]), 1)
                rows = feats[bounds[s] : bounds[s + 1]]
            else:
                rows = np.zeros((0, C), dtype=np.float32)
            in_maps.append(
                {
                    "x": _pack(rows, T, FD),
                    "invn": np.full((C, 1), 1.0 / n_j, dtype=np.float32),
                    "w": w,
                    "b": b,
                    "sel128": sel128,
                    "sel32": sel32,
                }
            )
        r = run_bass_kernel_spmd(nc, in_maps, list(range(NCORES)))
        global LAST_RESULTS
        LAST_RESULTS = r
        results = r.results
        for j, s in enumerate(gsegs):
            out[bounds[s] : bounds[s + 1]] = _unpack(results[j]["o"], int(counts[s]))
    return out


# revision 18
# speedup vs baseline: 2.6629x; 1.1748x over previous
"""MinkowskiInstanceNorm (segment-reduce instance norm) on 8 Trainium2 cores.

Strategy: seg_ids are sorted, so each segment is a contiguous run of rows.
With num_segments == n_cores == 8, core j owns segment j outright: it
computes sum(x) and sum(x^2) over its rows (padded to a fixed tile count
with zeros so one SPMD program serves all cores), derives
mean / inv_std / affine on-device, and normalizes in a second pass.
No cross-core communication is needed; the host only slices rows
per segment and stitches the outputs back in order.

Perf design (HBM-bound problem):
  - The device slab is fp16 (host casts on pack, upcasts on unpack),
    halving HBM traffic vs fp32.  Quantization error ~5e-4 relative,
    far inside the 2e-2 gate.
  - The whole per-core slab (T tiles of [128, FD]) stays RESIDENT in
    SBUF across both passes, so pass 2 reads from SBUF: HBM traffic is
    exactly one read + one write of the data.
  - Layout: channels on partitions — partition p = rb*32 + c (rb =
    row-block 0..3, c = channel), free axis = FD consecutive rows of
    that block.  Per-channel sums are full-free reductions; the
    normalize is one single-src DVE tensor_scalar (4x perf mode on
    fp16); cross-partition folding (4 row-blocks per channel) is a tiny
    matmul against a 0/1 selector, broadcast back via its transpose.
  - Pass-1 sum(x^2) is split between DVE (tensor_tensor_reduce) and ACT
    (Square activation with accum_out) so neither engine throttles the
    load stream; sum(x) is one DVE tensor_reduce per tile.
  - Loads ride the sync HWDGE ring, stores the ACT HWDGE ring.
"""

from contextlib import ExitStack

import numpy as np

C = 32  # channels
P = 128  # SBUF partitions
RB = P // C  # row blocks per tile (4)
NCORES = 8
EPS = 1e-8
T_TARGET = 8  # preferred tile count (keeps per-DMA size ~2 MiB)
FD_CAP = 8192  # largest free dim per tile we allow when growing T instead

_PROGRAMS = {}
LAST_RESULTS = None  # BassKernelResults of the most recent SPMD launch


def _cfg(maxc):
    """Pick (T, FD) so T*RB*FD >= maxc with small padding waste."""
    need = max(int(maxc), 1)
    fd = -(-need // (T_TARGET * RB * 64)) * 64
    fd = max(fd, 512)
    if fd <= FD_CAP:
        return T_TARGET, fd
    return -(-need // (RB * FD_CAP)), FD_CAP


def _emit(nc, tc, ctx, x_d, invn_d, w_d, b_d, s128_d, s16_d, s32_d, o_d, T, FD):
    from concourse import mybir

    dt = mybir.dt
    AX = mybir.AxisListType
    OP = mybir.AluOpType
    AF = mybir.ActivationFunctionType

    xv = x_d.ap()  # [T, P, FD]
    ov = o_d.ap()

    # Pass-1 engine assignment per tile:
    #   PE  — sum(x): selector matmuls accumulated in PSUM (~1.8us/tile)
    #   DVE — sum(x^2) over the leading DVE_F elems (stt, 1x ~1.07 ns/elem)
    #   ACT — sum(x^2) over the trailing ACT_F elems (Square+accum,
    #         ~0.886 ns/elem)
    # balanced so each engine stays under the ~4.8us/tile DMA rate.
    ACT_F = (int(FD * 0.549) // 32) * 32
    DVE_F = FD - ACT_F
    # PE fold geometry: 8 PSUM regions of CH columns; each tile's FD
    # columns hit each region twice (front/back half).
    NREG = 8
    CH = FD // (2 * NREG)
    assert FD % (2 * NREG) == 0 and CH * 4 <= 2048

    const = ctx.enter_context(tc.tile_pool(name="const", bufs=1))
    xpool = ctx.enter_context(tc.tile_pool(name="xpool", bufs=3))
    ypool = ctx.enter_context(tc.tile_pool(name="ypool", bufs=2))
    opool = ctx.enter_context(tc.tile_pool(name="opool", bufs=3))
    psum = ctx.enter_context(tc.tile_pool(name="psum", bufs=1, space="PSUM"))

    # Resident slab: as many tiles as fit in SBUF stay live across both
    # passes (all of them for the 2M-row / 8-segment problem).
    budget = 200 * 1024  # per-partition bytes
    fixed = 6 * FD + 2 * FD + 4096  # opool(3 bufs) + stt/ACT scratch + consts
    if T * FD * 2 + fixed <= budget:
        RES = T
    else:
        RES = max((budget - fixed - 10 * FD) // (FD * 2), 0)  # xpool+ypool
        RES = min(T, int(RES))
    res = const.tile([P, RES * FD], dt.float16, name="res") if RES else None
    ttrscr = const.tile([P, DVE_F], dt.float16)
    sqscr = const.tile([P, ACT_F], dt.float16, name="sqscr") if ACT_F else None
    drscr = const.tile([C, CH], dt.float32)

    qparts = const.tile([P, 2 * T], dt.float32)
    st_s = const.tile([C, NREG], dt.float32)

    # Issue every resident tile load first so the sync-ring FIFO streams
    # them back-to-back; consts ride the gpsimd (SWDGE) ring so sel16 is
    # ready before tile 0's first matmul.
    for i in range(RES):
        nc.sync.dma_start(out=res[:, i * FD : (i + 1) * FD], in_=xv[i])

    sel16 = const.tile([P, C], dt.float16)
    nc.gpsimd.dma_start(out=sel16[:], in_=s16_d.ap())
    invn = const.tile([C, 1], dt.float32)
    nc.gpsimd.dma_start(out=invn[:], in_=invn_d.ap())
    wt = const.tile([C, 1], dt.float32)
    nc.gpsimd.dma_start(out=wt[:], in_=w_d.ap())
    bt = const.tile([C, 1], dt.float32)
    nc.gpsimd.dma_start(out=bt[:], in_=b_d.ap())
    sel128 = const.tile([P, C], dt.float32)
    nc.gpsimd.dma_start(out=sel128[:], in_=s128_d.ap())
    sel32 = const.tile([C, P], dt.float32)
    nc.gpsimd.dma_start(out=sel32[:], in_=s32_d.ap())

    regs = [psum.tile([P, CH], dt.float32, name=f"ps{r}") for r in range(NREG)]

    HALF = FD // 2
    for i in range(T):
        if i < RES:
            xt = res[:, i * FD : (i + 1) * FD]
        else:
            xt = xpool.tile([P, FD], dt.float16, tag="xt")
            nc.sync.dma_start(out=xt[:], in_=xv[i])
        # sum(x): per-channel fold on the (otherwise idle) tensor engine;
        # PSUM region r accumulates both halves of every tile.
        for r in range(NREG):
            nc.tensor.matmul(
                regs[r][:C, :],
                lhsT=sel16[:],
                rhs=xt[:, r * CH : (r + 1) * CH],
                start=(i == 0),
                stop=False,
            )
            nc.tensor.matmul(
                regs[r][:C, :],
                lhsT=sel16[:],
                rhs=xt[:, HALF + r * CH : HALF + (r + 1) * CH],
                start=False,
                stop=(i == T - 1),
            )
        nc.vector.scalar_tensor_tensor(
            out=ttrscr[:],
            in0=xt[:, :DVE_F],
            scalar=1.0,
            in1=xt[:, :DVE_F],
            op0=OP.mult,
            op1=OP.mult,
            accum_out=qparts[:, i : i + 1],
        )
        if ACT_F:
            nc.scalar.activation(
                sqscr[:],
                xt[:, DVE_F:],
                AF.Square,
                accum_out=qparts[:, T + i : T + i + 1],
            )
        else:
            nc.vector.memset(qparts[:, T + i : T + i + 1], 0.0)

    # Drain the PSUM sum regions, split across DVE and ACT.
    for r in range(NREG):
        if r % 2 == 0:
            nc.vector.tensor_reduce(
                out=st_s[:, r : r + 1], in_=regs[r][:C, :], axis=AX.X, op=OP.add
            )
        else:
            nc.scalar.activation(
                drscr[:], regs[r][:C, :], AF.Copy,
                accum_out=st_s[:, r : r + 1],
            )
    ssum = const.tile([C, 1], dt.float32)
    nc.vector.tensor_reduce(out=ssum[:], in_=st_s[:], axis=AX.X, op=OP.add)

    qred = const.tile([P, 1], dt.float32)
    nc.vector.tensor_reduce(out=qred[:], in_=qparts[:], axis=AX.X, op=OP.add)
    # fold the RB row-blocks of each channel: [32, 1] = sel128.T @ qred
    nc.tensor.matmul(
        regs[0][:C, 0:1], lhsT=sel128[:], rhs=qred[:], start=True, stop=True
    )

    mean = const.tile([C, 1], dt.float32)
    nc.vector.tensor_scalar_mul(mean[:], ssum[:], invn[:])
    ex2 = const.tile([C, 1], dt.float32)
    nc.vector.tensor_scalar_mul(ex2[:], regs[0][:C, 0:1], invn[:])
    msq = const.tile([C, 1], dt.float32)
    nc.vector.tensor_mul(msq[:], mean[:], mean[:])
    var = const.tile([C, 1], dt.float32)
    nc.vector.tensor_sub(var[:], ex2[:], msq[:])
    epsv = const.tile([C, 1], dt.float32)
    nc.vector.memset(epsv[:], EPS)
    std = const.tile([C, 1], dt.float32)
    nc.scalar.activation(std[:], var[:], AF.Sqrt, bias=epsv[:])
    istd = const.tile([C, 1], dt.float32)
    nc.vector.reciprocal(istd[:], std[:])
    # ab = [A | B]: A = w/std, B = b - mean*A
    ab = const.tile([C, 2], dt.float32)
    nc.vector.tensor_mul(ab[:, 0:1], istd[:], wt[:])
    nc.vector.tensor_mul(ab[:, 1:2], mean[:], ab[:, 0:1])
    nc.vector.tensor_sub(ab[:, 1:2], bt[:], ab[:, 1:2])

    # broadcast A/B back to all 128 partitions: [128, 2] = sel32.T @ ab
    nc.tensor.matmul(
        regs[1][:, 0:2], lhsT=sel32[:], rhs=ab[:], start=True, stop=True
    )
    ab128 = const.tile([P, 2], dt.float32)
    nc.scalar.copy(ab128[:], regs[1][:, 0:2])

    for i in range(T):
        if i < RES:
            yt = res[:, i * FD : (i + 1) * FD]
        else:
            yt = ypool.tile([P, FD], dt.float16, tag="yt")
            nc.sync.dma_start(out=yt[:], in_=xv[i])
        ot = opool.tile([P, FD], dt.float16, tag="ot")
        nc.vector.tensor_scalar(
            out=ot[:],
            in0=yt[:],
            scalar1=ab128[:, 0:1],
            scalar2=ab128[:, 1:2],
            op0=OP.mult,
            op1=OP.add,
        )
        # alternate stores across both HWDGE rings (sync ring is idle in
        # pass 2) for better sustained store bandwidth
        if i % 2 == 0:
            nc.scalar.dma_start(out=ov[i], in_=ot[:])
        else:
            nc.sync.dma_start(out=ov[i], in_=ot[:])


def _get_program(T, FD):
    key = (T, FD)
    if key in _PROGRAMS:
        return _PROGRAMS[key]
    import concourse.tile as tile
    from concourse import bacc, mybir

    dt = mybir.dt
    nc = bacc.Bacc(
        "TRN2",
        target_bir_lowering=False,
        debug=False,
        enable_asserts=False,
        num_devices=NCORES,
    )
    x_d = nc.dram_tensor("x", [T, P, FD], dt.float16, kind="ExternalInput")
    invn_d = nc.dram_tensor("invn", [C, 1], dt.float32, kind="ExternalInput")
    w_d = nc.dram_tensor("w", [C, 1], dt.float32, kind="ExternalInput")
    b_d = nc.dram_tensor("b", [C, 1], dt.float32, kind="ExternalInput")
    s128_d = nc.dram_tensor("sel128", [P, C], dt.float32, kind="ExternalInput")
    s16_d = nc.dram_tensor("sel16", [P, C], dt.float16, kind="ExternalInput")
    s32_d = nc.dram_tensor("sel32", [C, P], dt.float32, kind="ExternalInput")
    o_d = nc.dram_tensor("o", [T, P, FD], dt.float16, kind="ExternalOutput")

    with tile.TileContext(nc) as tc:
        with ExitStack() as ctx:
            _emit(nc, tc, ctx, x_d, invn_d, w_d, b_d, s128_d, s16_d, s32_d, o_d, T, FD)

    nc.finalize()
    _PROGRAMS[key] = nc
    return nc


def _pack(rows, T, FD):
    """rows [n, C] f32 -> [T, 128, FD] f16: partition rb*32+c holds rows
    [t*RB*FD + rb*FD + j] of channel c at free index j; zero padded."""
    PAD = T * RB * FD
    xp = np.zeros((PAD, C), dtype=np.float16)
    xp[: rows.shape[0]] = rows
    return np.ascontiguousarray(
        xp.reshape(T, RB, FD, C).transpose(0, 1, 3, 2).reshape(T, P, FD)
    )


def _unpack(slab, n):
    """[T, 128, FD] f16 -> rows [n, C] f32."""
    T, _, FD = slab.shape
    return (
        slab.reshape(T, RB, C, FD)
        .transpose(0, 1, 3, 2)
        .reshape(T * RB * FD, C)[:n]
        .astype(np.float32)
    )


def kernel(feats, seg_ids, weight, bias, num_segments, **_):
    from concourse.bass_utils import run_bass_kernel_spmd

    feats = np.ascontiguousarray(np.asarray(feats), dtype=np.float32)
    seg = np.asarray(seg_ids)
    w = np.asarray(weight, dtype=np.float32).reshape(C, 1)
    b = np.asarray(bias, dtype=np.float32).reshape(C, 1)
    S = int(num_segments)
    N = feats.shape[0]

    assert (np.diff(seg) >= 0).all(), "seg_ids must be sorted"
    bounds = np.searchsorted(seg, np.arange(S + 1)).astype(np.int64)
    counts = np.diff(bounds)

    sel128 = np.ascontiguousarray(np.tile(np.eye(C, dtype=np.float32), (RB, 1)))
    sel32 = np.ascontiguousarray(sel128.T)

    out = np.empty((N, C), dtype=np.float32)
    for g0 in range(0, S, NCORES):
        gsegs = list(range(g0, min(g0 + NCORES, S)))
        maxc = max(int(counts[s]) for s in gsegs)
        T, FD = _cfg(maxc)
        nc = _get_program(T, FD)
        in_maps = []
        for j in range(NCORES):
            n_j = 1
            if j < len(gsegs):
                s = gsegs[j]
                n_j = max(int(counts[s]), 1)
                rows = feats[bounds[s] : bounds[s + 1]]
            else:
                rows = np.zeros((0, C), dtype=np.float32)
            in_maps.append(
                {
                    "x": _pack(rows, T, FD),
                    "invn": np.full((C, 1), 1.0 / n_j, dtype=np.float32),
                    "w": w,
                    "b": b,
                    "sel128": sel128,
                    "sel16": sel128.astype(np.float16),
                    "sel32": sel32,
                }
            )
        r = run_bass_kernel_spmd(nc, in_maps, list(range(NCORES)))
        global LAST_RESULTS
        LAST_RESULTS = r
        results = r.results
        for j, s in enumerate(gsegs):
            out[bounds[s] : bounds[s + 1]] = _unpack(results[j]["o"], int(counts[s]))
    return out
